# revision 21
# baseline (speedup 1.0000x reference)
"""CrossAttend Trainium2 kernel: 8-way data-parallel over batch.

Full inputs arrive here; we shard batch B=16 across 8 NeuronCores
(2 batch elements per core), replicate the 512x512 projection weights,
run one SPMD Bass/Tile kernel, and concatenate the per-core outputs.

Math notes (validated against the fp32 reference):
  - bk drops out entirely: it shifts every sim row by a constant per q,
    and softmax over k is shift-invariant.
  - qpk := qp @ Wk is shared by both attentions:
        sim  = qpk @ qp.T   (+ per-q const),   sim2 = qpk @ opp.T (+ const)
  - softmax is computed without max-subtraction (logits are O(5), exp is
    safe); the self-attention diagonal is zeroed after exp.
  - rowsums come from an extra N=4 matmul against a ones column that
    reuses the PE-resident P^T weights.

All matmuls run in bf16 (PSUM accumulation stays fp32): warm-PE bf16
streams 1 col/cycle @2.4GHz vs ~2x slower fp32r, and bf16 halves DMA.
q and opp are pre-transposed and cast on the host, so the kernel does
no PE transposes at all — activations arrive as [H, L] tiles ready to
be matmul operands, each landing via two ~1MB DMAs (16-way SDMA split).

On-chip layouts per batch element:
  qT, qpT, qpkT, oppT : [128, 4, 1024]  (h on partitions)
  v, opp_v            : [128, 8, 512]   (l on partitions)
  PexpT               : [128, 8, 1024]  (k on partitions, q free)

Engine budget: TensorE streams matmuls back-to-back (the bottleneck);
ScalarE does only PSUM evacuations (bias/exp/out-scale) so it never
delays a PSUM bank handoff; VectorE does the remaining evacuations and
reciprocals; Sync hosts input/output DMA issue; GpSimd hosts weight DMA
and the diagonal masks.
"""

import contextlib
import math

import ml_dtypes
import numpy as np

import concourse.bass as bass
import concourse.mybir as mybir
import concourse.tile as tile
from concourse import bacc
from concourse.bass_utils import run_bass_kernel_spmd

F32 = mybir.dt.float32
BF16 = mybir.dt.bfloat16

B = 16
H = 512
L = 1024
P = 128
NCORES = 8
BPC = B // NCORES   # batch elements per core
HT = H // P         # 4 h-tiles
LT = L // P         # 8 l-tiles
QC = L // 512       # 2 q-chunks of 512
SCALE = 1.0 / math.sqrt(H)


def _build_core_kernel(ctx, tc, ins, outs):
    nc = tc.nc
    AF = mybir.ActivationFunctionType

    qT_d = ins["qT"]        # [BPC, H, L] bf16 (host pre-transposed)
    oppT_d = ins["oppT"]    # [BPC, H, L] bf16
    self_d = outs["self_out"]
    oout_d = outs["opp_out"]

    wpool = ctx.enter_context(tc.tile_pool(name="w", bufs=1))
    big = ctx.enter_context(tc.tile_pool(name="big", bufs=5))
    vpool = ctx.enter_context(tc.tile_pool(name="v", bufs=2))
    ppool = ctx.enter_context(tc.tile_pool(name="P", bufs=2))
    opool = ctx.enter_context(tc.tile_pool(name="o", bufs=4))
    rpool = ctx.enter_context(tc.tile_pool(name="r", bufs=4))
    ps_mm = ctx.enter_context(tc.tile_pool(name="psmm", bufs=3, space="PSUM"))
    ps_sc = ctx.enter_context(tc.tile_pool(name="pssc", bufs=2, space="PSUM"))
    ps_rs = ctx.enter_context(tc.tile_pool(name="psrs", bufs=1, space="PSUM"))

    # --- PE warmup: ~6.5us of dummy matmuls with no DMA dependency, emitted
    # first so they hold the earliest PE queue slots.  They keep the PE busy
    # for the whole input-DMA ramp and flip the HAM clock-gate to K=8/8
    # (2.4GHz) before the real matmuls start.  Results go to a scratch PSUM
    # slice nobody reads.
    warm = wpool.tile([P, 512], BF16, tag="warm")
    nc.gpsimd.memset(warm[:], 0.0)
    pw = ps_rs.tile([P, 512], F32, tag="psrs")
    for _ in range(30):
        nc.tensor.matmul(pw[:], lhsT=warm[:, 0:128], rhs=warm[:],
                         start=True, stop=True)

    # --- constants (per-core replicated) ---
    wq = wpool.tile([P, HT, H], BF16, tag="wq")
    nc.gpsimd.dma_start(wq[:], ins["WqT"].rearrange("(ko ki) m -> ki ko m", ki=P))
    wk = wpool.tile([P, HT, H], BF16, tag="wk")
    nc.gpsimd.dma_start(wk[:], ins["Wk"].rearrange("(ko ki) m -> ki ko m", ki=P))
    wv = wpool.tile([P, HT, H], BF16, tag="wv")
    nc.gpsimd.dma_start(wv[:], ins["WvT"].rearrange("(ko ki) m -> ki ko m", ki=P))
    bq = wpool.tile([P, HT], F32, tag="bq")
    nc.gpsimd.dma_start(bq[:], ins["bq_p"][:])
    bvb = wpool.tile([P, H], F32, tag="bvb")
    nc.gpsimd.dma_start(bvb[:], ins["bv_b"][:])
    ones = wpool.tile([P, 4], BF16, tag="ones")
    nc.gpsimd.dma_start(ones[:], ins["ones_p"][:])

    def load_T(src_d, b):
        """Straight DMA of a host-pre-transposed [H, L] tensor into
        [128, HT, L]; two ~1MB DMAs so the first projection group (which
        only needs l-columns 0..511) starts after the first one."""
        xT = big.tile([P, HT, L], BF16, tag="big")
        src = src_d[b].rearrange("(ko ki) l -> ki ko l", ki=P)
        nc.sync.dma_start(xT[:, :, 0:512], src[:, :, 0:512])
        nc.sync.dma_start(xT[:, :, 512:L], src[:, :, 512:L])
        return xT

    def proj_T(src_T, w, bias=None):
        """dst[h_out-part, l] = sum_hin w[hin, hout-tile].T @ src_T[hin, l]."""
        dst = big.tile([P, HT, L], BF16, tag="big")
        for qc in range(QC):
            for ht in range(HT):
                ps = ps_mm.tile([P, 512], F32, tag="psmm")
                for hc in range(HT):
                    nc.tensor.matmul(
                        ps[:],
                        lhsT=w[:, hc, P * ht:P * (ht + 1)],
                        rhs=src_T[:, hc, 512 * qc:512 * (qc + 1)],
                        start=(hc == 0),
                        stop=(hc == HT - 1),
                    )
                d = dst[:, ht, 512 * qc:512 * (qc + 1)]
                if bias is not None:
                    nc.scalar.activation(d, ps[:], AF.Identity,
                                         bias=bias[:, ht:ht + 1], scale=1.0)
                else:
                    nc.vector.tensor_copy(d, ps[:])
        return dst

    def proj_nat(src_T, w_rhs, bias_b):
        """dst[l-part, h_out] = src_T[hin, l-tile].T @ w_rhs[hin, hout] + bias."""
        dst = vpool.tile([P, LT, H], BF16, tag="v")
        for lt in range(LT):
            ps = ps_mm.tile([P, 512], F32, tag="psmm")
            for hc in range(HT):
                nc.tensor.matmul(
                    ps[:],
                    lhsT=src_T[:, hc, P * lt:P * (lt + 1)],
                    rhs=w_rhs[:, hc, :],
                    start=(hc == 0),
                    stop=(hc == HT - 1),
                )
            nc.vector.tensor_tensor(dst[:, lt, :], ps[:], bias_b[:],
                                    mybir.AluOpType.add)
        return dst

    def scores(lhsT_T, qpkT, masked):
        """PexpT[k, q] = exp(scale * lhsT_T.T @ qpkT)."""
        pexp = ppool.tile([P, LT, L], BF16, tag="P")
        for ko in range(LT):
            ps = ps_sc.tile([P, L], F32, tag="pssc")
            for qc in range(QC):
                for hc in range(HT):
                    nc.tensor.matmul(
                        ps[:, 512 * qc:512 * (qc + 1)],
                        lhsT=lhsT_T[:, hc, P * ko:P * (ko + 1)],
                        rhs=qpkT[:, hc, 512 * qc:512 * (qc + 1)],
                        start=(hc == 0),
                        stop=(hc == HT - 1),
                    )
            # one 1024-wide exp amortizes the ACT fixed overhead (352 cyc)
            nc.scalar.activation(pexp[:, ko, :], ps[:], AF.Exp, scale=SCALE)
            if masked:
                qc = ko // (512 // P)
                m = ko % (512 // P)
                nc.gpsimd.affine_select(
                    out=pexp[:, ko, 512 * qc:512 * (qc + 1)],
                    in_=pexp[:, ko, 512 * qc:512 * (qc + 1)],
                    compare_op=mybir.AluOpType.not_equal,
                    fill=0.0, base=P * m,
                    pattern=[[-1, 512]], channel_multiplier=1,
                )
        return pexp

    def attn_av(pexp, vv, out_d, b):
        """out = (PexpT.T @ v) / rowsum, rowsum via ones matmul."""
        for qo in range(LT):
            pso = ps_mm.tile([P, 512], F32, tag="psmm")
            psr = ps_rs.tile([P, 4], F32, tag="psrs")
            for ko in range(LT):
                nc.tensor.matmul(
                    pso[:], lhsT=pexp[:, ko, P * qo:P * (qo + 1)],
                    rhs=vv[:, ko, :],
                    start=(ko == 0), stop=(ko == LT - 1),
                )
                nc.tensor.matmul(
                    psr[:], lhsT=pexp[:, ko, P * qo:P * (qo + 1)],
                    rhs=ones[:, 0:4],
                    start=(ko == 0), stop=(ko == LT - 1),
                )
            rc = rpool.tile([P, 1], F32, tag="r")
            nc.vector.reciprocal(rc[:], psr[:, 0:1])
            ot = opool.tile([P, 512], BF16, tag="o")
            nc.scalar.activation(ot[:], pso[:], AF.Copy, scale=rc[:, 0:1])
            nc.sync.dma_start(out_d[b, P * qo:P * (qo + 1), :], ot[:])

    for b in range(BPC):
        qT = load_T(qT_d, b)
        oppT = load_T(oppT_d, b)
        qpT = proj_T(qT, wq, bias=bq)
        qpkT = proj_T(qpT, wk)
        vv = proj_nat(qpT, wv, bvb)
        ovv = proj_nat(oppT, wv, bvb)
        pexp1 = scores(qpT, qpkT, masked=True)
        pexp2 = scores(oppT, qpkT, masked=False)
        attn_av(pexp1, vv, self_d, b)
        attn_av(pexp2, ovv, oout_d, b)


_NC_CACHE = None


def _get_module():
    global _NC_CACHE
    if _NC_CACHE is not None:
        return _NC_CACHE
    nc = bacc.Bacc(None, target_bir_lowering=False, debug=False)
    f32 = mybir.dt.float32
    bf16 = mybir.dt.bfloat16
    ins = {
        "qT": nc.dram_tensor("qT", [BPC, H, L], bf16, kind="ExternalInput").ap(),
        "oppT": nc.dram_tensor("oppT", [BPC, H, L], bf16,
                               kind="ExternalInput").ap(),
        "WqT": nc.dram_tensor("WqT", [H, H], bf16, kind="ExternalInput").ap(),
        "Wk": nc.dram_tensor("Wk", [H, H], bf16, kind="ExternalInput").ap(),
        "WvT": nc.dram_tensor("WvT", [H, H], bf16, kind="ExternalInput").ap(),
        "bq_p": nc.dram_tensor("bq_p", [P, HT], f32, kind="ExternalInput").ap(),
        "bv_b": nc.dram_tensor("bv_b", [P, H], f32, kind="ExternalInput").ap(),
        "ones_p": nc.dram_tensor("ones_p", [P, 4], bf16,
                                 kind="ExternalInput").ap(),
    }
    outs = {
        "self_out": nc.dram_tensor("self_out", [BPC, L, H], bf16,
                                   kind="ExternalOutput").ap(),
        "opp_out": nc.dram_tensor("opp_out", [BPC, L, H], bf16,
                                  kind="ExternalOutput").ap(),
    }
    with tile.TileContext(nc) as tc:
        with contextlib.ExitStack() as ctx:
            _build_core_kernel(ctx, tc, ins, outs)
    nc.compile()
    _NC_CACHE = nc
    return nc


def kernel(q, opp, Wq, bq, Wk, bk, Wv, bv):
    bf = ml_dtypes.bfloat16
    qT = np.ascontiguousarray(
        np.asarray(q, dtype=np.float32).astype(bf).transpose(0, 2, 1))
    oppT = np.ascontiguousarray(
        np.asarray(opp, dtype=np.float32).astype(bf).transpose(0, 2, 1))
    Wq = np.asarray(Wq, dtype=np.float32)
    Wk = np.asarray(Wk, dtype=np.float32)
    Wv = np.asarray(Wv, dtype=np.float32)
    bq = np.asarray(bq, dtype=np.float32)
    bv = np.asarray(bv, dtype=np.float32)
    # bk is mathematically irrelevant (softmax shift-invariance); unused.

    shared = {
        "WqT": np.ascontiguousarray(Wq.T.astype(bf)),
        "Wk": np.ascontiguousarray(Wk.astype(bf)),
        "WvT": np.ascontiguousarray(Wv.T.astype(bf)),
        "bq_p": np.ascontiguousarray(bq.reshape(HT, P).T),
        "bv_b": np.ascontiguousarray(np.tile(bv, (P, 1))),
        "ones_p": np.ones((P, 4), dtype=bf),
    }
    in_maps = []
    for c in range(NCORES):
        sl = slice(c * BPC, (c + 1) * BPC)
        in_maps.append({
            "qT": np.ascontiguousarray(qT[sl]),
            "oppT": np.ascontiguousarray(oppT[sl]),
            **shared,
        })

    nc = _get_module()
    res = run_bass_kernel_spmd(nc, in_maps, core_ids=list(range(NCORES)))
    self_out = np.concatenate(
        [r["self_out"].astype(np.float32) for r in res.results], axis=0)
    opp_out = np.concatenate(
        [r["opp_out"].astype(np.float32) for r in res.results], axis=0)
    return (self_out, opp_out)
